# revision 1
# baseline (speedup 1.0000x reference)
"""Cross-attention (GQA, key-padding + shifted-causal mask) on 8 Trainium2 cores.

Sharding: core k handles batch b = k//4 and kv heads {2*(k%4), 2*(k%4)+1}
(each with its 4 query heads under GQA) -> 8 (b,h) attention instances per
core, fully independent (no collectives).

Mask algebra: the reference adds -10000 for padded keys and *replaces* with
-10000 where s > t + len_b - Sk. Since len_b >= Sk/2, the causal condition
subsumes the padding one, so the effective rule is "key s visible to query t
iff s <= t - c_b" with c_b = Sk - len_b. Rolling K/V right by c_b turns this
into a standard causal mask (s' <= t), which is compile-time structure: the
same SPMD program works for any lengths. Rolled-in garbage rows (s' < c_b)
are neutralized by zeroed V rows and an m_pad-weighted denominator matmul.
Rows with t < c_b attend to nothing; the reference gives them a uniform
softmax (all scores equal -10000), i.e. mean(V) -- patched on host.

Per (b,h) the device computes, in score-transposed (ST) layout [s, t]:
  ST = (K'^T)^T @ Q^T      (f32r matmuls, 128-wide s blocks x 512-wide t)
  P  = exp(scale * ST)     (ScalarE, skipping blocks above the causal diag)
  P *= diag_mask           (only on the 4 diagonal block columns, VectorE)
  OT   = sum_s V'[s,d] P[s,t]        (PSUM accum over s blocks)
  den  = sum_s m_pad_rep[s,m] P[s,t] (same, gives den broadcast over m)
  out  = OT * 1/(den + eps)          (VectorE), stored d-major; host
                                      transposes back to (B, Sq, H, D).
"""

import numpy as np

B, SQ, SK, H, HK, D = 2, 2048, 2048, 32, 8, 128
G = H // HK            # query heads per kv head
N_CORES = 8
TQ = 512               # t (query) tile width
TS = 128               # s (key) tile width
NTQ = SQ // TQ         # 4 t-chunks
SCALE = 1.0 / float(np.sqrt(D))
DEN_EPS = 1e-30

_compiled = None


def _build_program():
    """Build + schedule the single SPMD Bass program (same for all cores)."""
    from contextlib import ExitStack
    import concourse.bass as bass
    import concourse.tile as tile
    from concourse import bacc, mybir

    f32 = mybir.dt.float32
    f32r = mybir.dt.float32r

    nc = bacc.Bacc("TRN2", target_bir_lowering=False, debug=False)
    qT_ap = nc.dram_tensor("qT", [2 * G, D, SQ], f32, kind="ExternalInput").ap()
    kT_ap = nc.dram_tensor("kT", [2, D, SK], f32, kind="ExternalInput").ap()
    v_ap = nc.dram_tensor("v", [2, TS, SK // TS * D], f32, kind="ExternalInput").ap()
    mpr_ap = nc.dram_tensor("mpr", [TS, SK], f32, kind="ExternalInput").ap()
    out_ap = nc.dram_tensor("out", [2 * G, D, SQ], f32, kind="ExternalOutput").ap()

    with tile.TileContext(nc) as tc, ExitStack() as ctx:
        const_pool = ctx.enter_context(tc.tile_pool(name="const", bufs=1))
        kv_pool = ctx.enter_context(tc.tile_pool(name="kv", bufs=2))
        q_pool = ctx.enter_context(tc.tile_pool(name="q", bufs=2))
        p_pool = ctx.enter_context(tc.tile_pool(name="p", bufs=4))
        fin_pool = ctx.enter_context(tc.tile_pool(name="fin", bufs=3))
        st_psum = ctx.enter_context(tc.tile_pool(name="st", bufs=2, space="PSUM"))
        ot_psum = ctx.enter_context(tc.tile_pool(name="ot", bufs=2, space="PSUM"))
        den_psum = ctx.enter_context(tc.tile_pool(name="den", bufs=2, space="PSUM"))

        mpr_sb = const_pool.tile([TS, SK], f32r)

        for ikv in range(2):
            kT_sb = kv_pool.tile([D, SK], f32r, tag="kT")
            v_sb = kv_pool.tile([TS, SK // TS * D], f32r, tag="v")
            if ikv == 0:
                # startup: order sync queue by first consumption, park the
                # not-immediately-needed loads on the idle gpsimd queue
                nc.sync.dma_start(kT_sb[:, :TQ], kT_ap[ikv][:, :TQ].bitcast(f32r))
                nc.gpsimd.dma_start(v_sb[:], v_ap[ikv].bitcast(f32r))
                nc.gpsimd.dma_start(mpr_sb[:], mpr_ap[:].bitcast(f32r))
            else:
                nc.sync.dma_start(kT_sb[:], kT_ap[ikv].bitcast(f32r))
                nc.sync.dma_start(v_sb[:], v_ap[ikv].bitcast(f32r))

            for j in range(G):
                ih = ikv * G + j
                qT_sb = q_pool.tile([D, SQ], f32r)
                if ikv == 0 and j == 0:
                    nc.sync.dma_start(qT_sb[:, :TQ], qT_ap[ih][:, :TQ].bitcast(f32r))
                    nc.sync.dma_start(qT_sb[:, TQ:], qT_ap[ih][:, TQ:].bitcast(f32r))
                    nc.sync.dma_start(kT_sb[:, TQ:], kT_ap[ikv][:, TQ:].bitcast(f32r))
                else:
                    nc.sync.dma_start(qT_sb[:], qT_ap[ih].bitcast(f32r))

                for t in range(NTQ):
                    n_sc = (TQ // TS) * (t + 1)  # causal: s blocks up to diag
                    ot_ps = ot_psum.tile([D, TQ], f32)
                    den_ps = den_psum.tile([TS, TQ], f32)
                    pending = None  # 1-deep SW pipeline keeps PE ahead of ACT
                    for pi in range(n_sc // 2):
                        sc0 = 2 * pi
                        st_ps = st_psum.tile([TS, 2 * TQ], f32)
                        for h in range(2):
                            nc.tensor.matmul(
                                st_ps[:, h * TQ : (h + 1) * TQ],
                                lhsT=kT_sb[:, (sc0 + h) * TS : (sc0 + h + 1) * TS],
                                rhs=qT_sb[:, t * TQ : (t + 1) * TQ],
                                start=True,
                                stop=True,
                            )
                        p_sb = p_pool.tile([TS, 2 * TQ], f32r)
                        nc.scalar.activation(
                            p_sb[:], st_ps[:],
                            mybir.ActivationFunctionType.Exp,
                            scale=SCALE,
                        )
                        for h in range(2):
                            o = sc0 + h - (n_sc - 4)
                            if o >= 0:  # diagonal block: causal pattern
                                nc.gpsimd.affine_select(
                                    out=p_sb[:, h * TQ : (h + 1) * TQ],
                                    in_=p_sb[:, h * TQ : (h + 1) * TQ],
                                    pattern=[[1, TQ]],
                                    compare_op=mybir.AluOpType.is_ge,
                                    fill=0.0,
                                    base=-o * TS,
                                    channel_multiplier=-1,
                                )
                        if pending is not None:
                            _pv_den(nc, pending, v_sb, mpr_sb, ot_ps, den_ps,
                                    first=(pending[0] == 0), last=False)
                        pending = (sc0, p_sb)
                    _pv_den(nc, pending, v_sb, mpr_sb, ot_ps, den_ps,
                            first=(pending[0] == 0), last=True,
                            n_sc=n_sc)

                    recip_sb = fin_pool.tile([TS, TQ], f32, tag="recip")
                    nc.vector.reciprocal_approx_fast(recip_sb[:], den_ps[:])
                    out_sb = fin_pool.tile([D, TQ], f32, tag="out")
                    nc.vector.tensor_tensor(
                        out=out_sb[:],
                        in0=ot_ps[:],
                        in1=recip_sb[:],
                        op=mybir.AluOpType.mult,
                    )
                    nc.sync.dma_start(
                        out_ap[ih][:, t * TQ : (t + 1) * TQ], out_sb[:]
                    )

    nc.compile()
    return nc


def _pv_den(nc, pending, v_sb, mpr_sb, ot_ps, den_ps, first, last, n_sc=None):
    sc0, p_sb = pending
    for h in range(2):
        sc = sc0 + h
        nc.tensor.matmul(
            ot_ps[:],
            lhsT=v_sb[:, sc * D : (sc + 1) * D],
            rhs=p_sb[:, h * TQ : (h + 1) * TQ],
            start=(first and h == 0),
            stop=(last and h == 1),
        )
        nc.tensor.matmul(
            den_ps[:],
            lhsT=mpr_sb[:, sc * TS : (sc + 1) * TS],
            rhs=p_sb[:, h * TQ : (h + 1) * TQ],
            start=(first and h == 0),
            stop=(last and h == 1),
        )


def _get_program():
    global _compiled
    if _compiled is None:
        _compiled = _build_program()
    return _compiled


def kernel(q, kv, key_padding_mask, _want_trace=False):
    q = np.asarray(q, dtype=np.float32)
    kv = np.asarray(kv, dtype=np.float32)
    mask = np.asarray(key_padding_mask).astype(bool)

    lengths = mask.sum(axis=1).astype(np.int64)  # valid keys per batch
    c = SK - lengths                             # roll shift per batch

    k_full = kv[:, :, 0]  # (B, SK, HK, D)
    v_full = kv[:, :, 1]

    # roll keys/values right by c[b]; only the first len_b keys are ever
    # visible so the tail [len_b:] is dropped. Pad region stays zero.
    k_roll = np.zeros_like(k_full)
    v_roll = np.zeros_like(v_full)
    for b in range(B):
        k_roll[b, c[b]:] = k_full[b, : lengths[b]]
        v_roll[b, c[b]:] = v_full[b, : lengths[b]]

    in_maps = []
    for core in range(N_CORES):
        b = core // 4
        hks = (2 * (core % 4), 2 * (core % 4) + 1)
        qT = np.empty((2 * G, D, SQ), dtype=np.float32)
        kT = np.empty((2, D, SK), dtype=np.float32)
        v_l = np.empty((2, TS, SK // TS * D), dtype=np.float32)
        for i, hk in enumerate(hks):
            kT[i] = k_roll[b, :, hk, :].T
            # v chunked: v_l[i][p, sc*D + d] = v_roll[b, sc*TS + p, hk, d]
            v_l[i] = np.ascontiguousarray(
                v_roll[b, :, hk, :].reshape(SK // TS, TS, D).transpose(1, 0, 2)
            ).reshape(TS, SK // TS * D)
            for j in range(G):
                qT[i * G + j] = q[b, :, hk * G + j, :].T
        mpad = (np.arange(SK) >= c[b]).astype(np.float32)
        # mpr[p, sc*TS + m] = mpad[sc*TS + p]  (column-replicated per chunk)
        mpr = np.repeat(
            mpad.reshape(SK // TS, TS, 1), TS, axis=2
        ).transpose(1, 0, 2).reshape(TS, SK).astype(np.float32)
        in_maps.append({
            "qT": np.ascontiguousarray(qT),
            "kT": np.ascontiguousarray(kT),
            "v": np.ascontiguousarray(v_l),
            "mpr": np.ascontiguousarray(mpr),
        })

    from concourse.bass_utils import run_bass_kernel_spmd

    nc = _get_program()
    res = run_bass_kernel_spmd(
        nc, in_maps, core_ids=list(range(N_CORES)),
        trace=_want_trace,
    )

    out = np.empty((B, SQ, H, D), dtype=np.float32)
    for core in range(N_CORES):
        b = core // 4
        hks = (2 * (core % 4), 2 * (core % 4) + 1)
        o_core = res.results[core]["out"]  # (2*G, D, SQ)
        for i, hk in enumerate(hks):
            for j in range(G):
                out[b, :, hk * G + j, :] = o_core[i * G + j].T

    # rows that attend to nothing: reference softmax is uniform -> mean(V)
    for b in range(B):
        if c[b] > 0:
            vm = v_full[b].mean(axis=0)  # (HK, D)
            out[b, : c[b]] = np.repeat(vm, G, axis=0)[None]

    if _want_trace:
        return out, res
    return out



# revision 6
# speedup vs baseline: 1.6962x; 1.6962x over previous
"""Cross-attention (GQA, key-padding + shifted-causal mask) on 8 Trainium2 cores.

Sharding: core k handles kv head k for BOTH batches (4 query heads each under
GQA) -> 8 (b,h) attention instances per core, no collectives. This balances
work across cores because per-batch work depends on the ragged length.

Mask algebra: the reference adds -10000 for padded keys and replaces with
-10000 where s > t + len_b - Sk. With c_b = Sk - len_b the effective rule is
"key s visible to query t iff s <= t - c_b" (causality subsumes padding since
t - c_b <= len_b - 1 always). So per query chunk only the PREFIX of s-blocks
up to the causal diagonal participates; c_b is read from the runtime mask and
the program is compiled per (c_0, c_1) (cached). Queries t < c_b attend to
nothing; the reference gives them a uniform softmax -> mean(V), patched on
host.

Per (b,h), in score-transposed layout [s, t] with TQ=256 query chunks:
  ST = K^T Q           (bf16 matmuls, one per 128-wide s block)
  P  = exp(scale*ST)   (ScalarE, grouped up to 4 s-blocks per call, fp16 out)
  P *= diag_mask       (gpsimd affine_select on partially-masked blocks only)
  OT   += V'[s,d] P[s,t]   (fp16 matmuls, PSUM accum over s blocks)
  Pacc += P                (VectorE fp16 accumulate across s blocks)
  denT[m] = sum_p Pacc[p, m]  (two tiny [128,1]-out matmuls vs an all-ones rhs)
  OT, denT -> SBUF -> DRAM; host computes OT/den and transposes to
  (B, Sq, H, D).
"""

import numpy as np

B, SQ, SK, H, HK, D = 2, 2048, 2048, 32, 8, 128
G = H // HK            # query heads per kv head
N_CORES = 8
TQ = 256               # t (query) tile width
TS = 128               # s (key) tile width
NTQ = SQ // TQ         # 8 t-chunks
GRP = 4                # s-blocks per exp group / ST psum tile
SCALE = 1.0 / float(np.sqrt(D))

_compiled = {}


def _nb_table(c):
    """Number of s-blocks per t-chunk for shift c (prefix up to causal diag)."""
    nbmax = (SK - 1 - c) // TS + 1
    out = []
    for tc in range(NTQ):
        nb = (TQ * tc + TQ - 1 - c) // TS + 1
        out.append(min(max(nb, 0), nbmax))
    return out


def _build_program(c):
    """Build + schedule the SPMD Bass program, specialized on (c0, c1)."""
    from contextlib import ExitStack
    import concourse.bass as bass
    import concourse.tile as tile
    from concourse import bacc, mybir

    f32 = mybir.dt.float32
    bf16 = mybir.dt.bfloat16
    f16 = mybir.dt.float16

    nb_tabs = [_nb_table(ci) for ci in c]
    nbmaxs = [(SK - 1 - ci) // TS + 1 for ci in c]
    NBK = max(nbmaxs)

    nc = bacc.Bacc("TRN2", target_bir_lowering=False, debug=False)
    qT_ap = nc.dram_tensor("qT", [2 * G, D, SQ], bf16, kind="ExternalInput").ap()
    kT_ap = nc.dram_tensor("kT", [2, D, NBK * TS], bf16, kind="ExternalInput").ap()
    v_ap = nc.dram_tensor("v", [2, TS, NBK * D], f16, kind="ExternalInput").ap()
    out_ap = nc.dram_tensor("out", [2 * G, D, SQ], f32, kind="ExternalOutput").ap()
    den_ap = nc.dram_tensor("den", [2 * G, TS, 2 * NTQ], f32,
                            kind="ExternalOutput").ap()

    with tile.TileContext(nc) as tc, ExitStack() as ctx:
        const_pool = ctx.enter_context(tc.tile_pool(name="const", bufs=1))
        kv_pool = ctx.enter_context(tc.tile_pool(name="kv", bufs=2))
        q_pool = ctx.enter_context(tc.tile_pool(name="q", bufs=2))
        p_pool = ctx.enter_context(tc.tile_pool(name="p", bufs=4))
        pacc_pool = ctx.enter_context(tc.tile_pool(name="pacc", bufs=2))
        osb_pool = ctx.enter_context(tc.tile_pool(name="osb", bufs=3))
        densb_pool = ctx.enter_context(tc.tile_pool(name="densb", bufs=2))
        st_psum = ctx.enter_context(tc.tile_pool(name="st", bufs=2, space="PSUM"))
        ot_psum = ctx.enter_context(tc.tile_pool(name="ot", bufs=2, space="PSUM"))
        den_psum = ctx.enter_context(tc.tile_pool(name="den", bufs=2, space="PSUM"))

        ones_sb = const_pool.tile([TS, 1], f16)
        nc.vector.memset(ones_sb[:], 1.0)

        pending = None  # 1-deep SW pipeline keeps PE ahead of ACT

        def flush(pend):
            # PV matmuls for a finished group; on the chunk's last group also
            # emit denT, PSUM->SBUF copies and the output DMAs.
            for u in range(pend["gn"]):
                sc = pend["g0"] + u
                nc.tensor.matmul(
                    pend["ot_ps"][:],
                    lhsT=pend["v_sb"][:, sc * D : (sc + 1) * D],
                    rhs=pend["p_sb"][:, u * TQ : (u + 1) * TQ],
                    start=(pend["first"] and u == 0),
                    stop=(pend["last"] and u == pend["gn"] - 1),
                )
            if pend["last"]:
                den_ps = den_psum.tile([TS, 2], f32)
                for half in range(2):
                    nc.tensor.matmul(
                        den_ps[:, half : half + 1],
                        lhsT=pend["pacc"][:, half * TS : (half + 1) * TS],
                        rhs=ones_sb[:, 0:1],
                        start=True,
                        stop=True,
                    )
                osb = osb_pool.tile([D, TQ], f32)
                nc.vector.tensor_copy(out=osb[:], in_=pend["ot_ps"][:])
                nc.vector.tensor_copy(
                    out=pend["den_sb"][:, 2 * pend["tc"] : 2 * pend["tc"] + 2],
                    in_=den_ps[:],
                )
                nc.sync.dma_start(
                    out_ap[pend["ih"]][:, pend["tc"] * TQ : (pend["tc"] + 1) * TQ],
                    osb[:],
                )
                if pend["last_of_inst"]:
                    nc.gpsimd.dma_start(den_ap[pend["ih"]], pend["den_sb"][:])

        for i in range(2):  # batch
            cb = c[i]
            nbk = nbmaxs[i]
            nb_tab = nb_tabs[i]
            kT_sb = kv_pool.tile([D, NBK * TS], bf16, tag="kT")
            v_sb = kv_pool.tile([TS, NBK * D], f16, tag="v")
            nc.sync.dma_start(kT_sb[:, : nbk * TS], kT_ap[i][:, : nbk * TS])
            nc.gpsimd.dma_start(v_sb[:, : nbk * D], v_ap[i][:, : nbk * D])

            for j in range(G):
                ih = i * G + j
                qT_sb = q_pool.tile([D, SQ], bf16)
                nc.sync.dma_start(qT_sb[:], qT_ap[ih])
                den_sb = densb_pool.tile([TS, 2 * NTQ], f32)
                nc.gpsimd.memset(den_sb[:], 0.0)

                for tcix in range(NTQ):
                    nb = nb_tab[tcix]
                    if nb == 0:
                        continue
                    ot_ps = ot_psum.tile([D, TQ], f32)
                    pacc = pacc_pool.tile([TS, TQ], f16)
                    g0 = 0
                    while g0 < nb:
                        gn = min(GRP, nb - g0)
                        st_ps = st_psum.tile([TS, GRP * TQ], f32)
                        for u in range(gn):
                            sc = g0 + u
                            nc.tensor.matmul(
                                st_ps[:, u * TQ : (u + 1) * TQ],
                                lhsT=kT_sb[:, sc * TS : (sc + 1) * TS],
                                rhs=qT_sb[:, tcix * TQ : (tcix + 1) * TQ],
                                start=True,
                                stop=True,
                            )
                        p_sb = p_pool.tile([TS, GRP * TQ], f16)
                        nc.scalar.activation(
                            p_sb[:, : gn * TQ], st_ps[:, : gn * TQ],
                            mybir.ActivationFunctionType.Exp,
                            scale=SCALE,
                        )
                        for u in range(gn):
                            sc = g0 + u
                            bv = TS * sc + cb - TQ * tcix
                            if bv > -(TS - 1):  # partially masked block
                                nc.gpsimd.affine_select(
                                    out=p_sb[:, u * TQ : (u + 1) * TQ],
                                    in_=p_sb[:, u * TQ : (u + 1) * TQ],
                                    pattern=[[1, TQ]],
                                    compare_op=mybir.AluOpType.is_ge,
                                    fill=0.0,
                                    base=-bv,
                                    channel_multiplier=-1,
                                )
                        for u in range(gn):
                            if g0 + u == 0:
                                nc.vector.tensor_copy(
                                    out=pacc[:], in_=p_sb[:, :TQ])
                            else:
                                nc.vector.tensor_tensor(
                                    out=pacc[:],
                                    in0=pacc[:],
                                    in1=p_sb[:, u * TQ : (u + 1) * TQ],
                                    op=mybir.AluOpType.add,
                                )
                        if pending is not None:
                            flush(pending)
                        pending = {
                            "g0": g0, "gn": gn, "p_sb": p_sb, "v_sb": v_sb,
                            "ot_ps": ot_ps, "pacc": pacc, "den_sb": den_sb,
                            "ih": ih, "tc": tcix,
                            "first": g0 == 0, "last": g0 + gn >= nb,
                            "last_of_inst": (g0 + gn >= nb
                                             and tcix == NTQ - 1),
                        }
                        g0 += gn

        if pending is not None:
            flush(pending)

    nc.compile()
    return nc


def _get_program(c):
    key = tuple(int(x) for x in c)
    if key not in _compiled:
        _compiled[key] = _build_program(key)
    return _compiled[key]


def kernel(q, kv, key_padding_mask, _want_trace=False):
    import ml_dtypes

    bf16 = ml_dtypes.bfloat16
    q = np.asarray(q, dtype=np.float32)
    kv = np.asarray(kv, dtype=np.float32)
    mask = np.asarray(key_padding_mask).astype(bool)

    lengths = mask.sum(axis=1).astype(np.int64)
    # contiguous-prefix masks assumed (reference builds them that way)
    assert all(mask[b, : lengths[b]].all() and not mask[b, lengths[b]:].any()
               for b in range(B))
    c = tuple(int(SK - l) for l in lengths)
    nbmaxs = [(SK - 1 - ci) // TS + 1 for ci in c]
    NBK = max(nbmaxs)

    k_full = kv[:, :, 0]  # (B, SK, HK, D)
    v_full = kv[:, :, 1]

    k_bf = k_full.astype(bf16)
    v_16 = v_full.astype(np.float16)
    q_bf = q.astype(bf16)

    in_maps = []
    for core in range(N_CORES):
        hk = core
        qT = np.empty((2 * G, D, SQ), dtype=bf16)
        kT = np.zeros((2, D, NBK * TS), dtype=bf16)
        v_l = np.zeros((2, TS, NBK * D), dtype=np.float16)
        for i in range(B):
            nbk = nbmaxs[i]
            kT[i, :, : nbk * TS] = k_bf[i, : nbk * TS, hk, :].T
            # v chunked: v_l[i][p, sc*D + d] = v[i, sc*TS + p, hk, d]
            v_l[i, :, : nbk * D] = np.ascontiguousarray(
                v_16[i, : nbk * TS, hk, :].reshape(nbk, TS, D).transpose(1, 0, 2)
            ).reshape(TS, nbk * D)
            for j in range(G):
                qT[i * G + j] = q_bf[i, :, hk * G + j, :].T
        in_maps.append({
            "qT": np.ascontiguousarray(qT),
            "kT": kT,
            "v": v_l,
        })

    from concourse.bass_utils import run_bass_kernel_spmd

    nc = _get_program(c)
    res = run_bass_kernel_spmd(
        nc, in_maps, core_ids=list(range(N_CORES)),
        trace=_want_trace,
    )

    out = np.empty((B, SQ, H, D), dtype=np.float32)
    for core in range(N_CORES):
        hk = core
        o_core = res.results[core]["out"]    # (2*G, D, SQ) f32
        den_core = res.results[core]["den"]  # (2*G, TS, 2*NTQ) f32
        for i in range(B):
            for j in range(G):
                ih = i * G + j
                # den[p, 2*tc+half] -> t = tc*TQ + half*TS + p
                den_t = den_core[ih].reshape(TS, NTQ, 2).transpose(1, 2, 0)
                den_t = den_t.reshape(SQ)
                with np.errstate(divide="ignore", invalid="ignore"):
                    out[i, :, hk * G + j, :] = (o_core[ih] / den_t[None, :]).T

    # rows that attend to nothing: reference softmax is uniform -> mean(V)
    for b in range(B):
        if c[b] > 0:
            vm = v_full[b].mean(axis=0)  # (HK, D)
            out[b, : c[b]] = np.repeat(vm, G, axis=0)[None]

    if _want_trace:
        return out, res
    return out
